# revision 46
# baseline (speedup 1.0000x reference)
"""Trainium2 Bass kernel for the AttentionLayer problem.

Math (per batch):
    Q = inp_q @ Wq + bq            [S, d]
    K = inp_k @ Wk + bk            [S, d]
    V = inp_v @ Wv + bv            [S, d]
    sc = Q @ K^T / sqrt(d)         [Sq, Sk]
    S_ = softmax(sc, axis=0)       (over the QUERY axis)
    H = S_ @ V                     [Sq, d]

Device-side layout strategy (per core, 2 batches):
  * Host feeds transposed activations xT = x^T [D, S] in bf16 so every
    matmul contracts over the SBUF partition dim with zero on-chip
    transposes and minimal HBM traffic (compute is bf16 anyway).
  * Projections produce QT/KT in [d, S] layout (d = 128 partitions).
  * scores^T [k, q] = (KT-slice)^T @ QT, so softmax-over-q is a
    free-axis row reduction.  No max-subtraction is needed:
    |sc/sqrt(d)| <~ 6 for randn inputs, exp() is exact in f32 there.
  * The scores chain is ACT-bound (exp of S^2 elements), so all other
    PE work is interleaved INTO it: K-slab projections (double-buffered
    PSUM bank pair) and the V projection, computed in natural [S, d]
    layout as 4-strip bank groups once all four V chunks have landed.
  * Z[k] = sum_q exp is a DVE reduce over the bf16 pt tile for most
    chunks (cheaper than ACT's accum-readout); the last 4 chunks keep
    the ACT accumulator so the AV phase isn't gated on tail reduces.
  * Normalization is folded into V: vs[k, :] = V[k, :] / Z[k], then
    H^T [d, q] += vs-slice^T @ P^T accumulates per 512-col strip in a
    single PSUM bank; each strip converts to bf16 and DMAs out while
    the next strip's matmuls run.
  * PSUM budget: scores double-buffer 2x[128,1024] (4 banks) + K-slab
    pair (2 banks) + V/AV strip pair (2 banks) = 8 banks.
  * DMA triggers: all input loads on gpsimd in program order (the DMA
    bus is FIFO by trigger time; one queue keeps big transfers from
    cutting ahead of later-needed slabs), weights on scalar, biases +
    output on sync.  Descriptor generation costs ~1us fixed per
    dma_start, so parallel queues matter at startup.
  * Output H^T stored bf16; host upcasts + un-transposes.
Compute dtype bf16 (f32 PSUM accumulate), stats in f32.
"""

import math
import sys

sys.path.insert(0, "/opt/trn_rl_repo")

import ml_dtypes
import numpy as np

import concourse.bass as bass  # noqa: E402
import concourse.tile as tile  # noqa: E402
from concourse import bacc, mybir  # noqa: E402

P = 128          # partitions / head dim d
S = 2048         # sequence length
D = 1024         # model dim
DC = D // P      # D chunks (8)
KC = S // P      # key chunks (16)
B_LOC = 2        # batches per core
N_CORES = 8
SCALE = 1.0 / math.sqrt(P)
N_HYB = 4        # trailing k-chunks whose Z uses the ACT accumulator

F32 = mybir.dt.float32
BF16 = mybir.dt.bfloat16

_BUILT = None  # cached (nc,) so repeated kernel() calls reuse the NEFF


def build():
    nc = bacc.Bacc("TRN2", target_bir_lowering=False, debug=False,
                   num_devices=N_CORES)

    dr_in = {}
    for t in ("q", "k", "v"):
        dr_in[t] = nc.dram_tensor(f"{t}T", [B_LOC, D, S], BF16,
                                  kind="ExternalInput")
    dr_w = {t: nc.dram_tensor(f"w{t}", [D, P], BF16, kind="ExternalInput")
            for t in ("q", "k", "v")}
    dr_b = {t: nc.dram_tensor(f"b{t}", [P], F32, kind="ExternalInput")
            for t in ("q", "k", "v")}
    dr_out = nc.dram_tensor("out", [B_LOC, P, S], BF16,
                            kind="ExternalOutput")

    with tile.TileContext(nc) as tc:
        with (
            tc.tile_pool(name="const", bufs=1) as const,
            tc.tile_pool(name="streama", bufs=8) as streama,
            tc.tile_pool(name="streamk", bufs=8) as streamk,
            tc.tile_pool(name="proj", bufs=2) as proj,
            tc.tile_pool(name="kctp", bufs=10) as kctp,
            tc.tile_pool(name="ptp", bufs=16) as ptp,
            tc.tile_pool(name="vsp", bufs=18) as vsp,
            tc.tile_pool(name="recp", bufs=18) as recp,
            tc.tile_pool(name="zzp", bufs=6) as zzp,
            tc.tile_pool(name="osb", bufs=1) as osb,
            tc.tile_pool(name="ps_big", bufs=2, space="PSUM") as ps_big,
            tc.tile_pool(name="ps_kps", bufs=2, space="PSUM") as ps_kps,
            tc.tile_pool(name="ps_out", bufs=2, space="PSUM") as ps_out,
        ):
            w_sb = {}
            b_sb = {}
            _w_loaded = set()

            def ensure_w(t):
                if t in _w_loaded:
                    return
                _w_loaded.add(t)
                nc.scalar.dma_start(
                    w_sb[t][:],
                    dr_w[t].ap().rearrange("(c p) e -> p c e", p=P))

            for t in ("q", "k", "v"):
                w_sb[t] = const.tile([P, DC, P], BF16, tag=f"w{t}",
                                     name=f"w{t}")
                b_sb[t] = const.tile([P, 1], F32, tag=f"b{t}", name=f"b{t}")
                nc.sync.dma_start(
                    b_sb[t][:],
                    dr_b[t].ap().rearrange("(p o) -> p o", o=1))
            # V bias as a rank-1 matmul (ones[1,128].T @ bias_row[1,128])
            # closing each V strip's accumulation group; created lazily
            _vbias_box = []

            def ensure_vbias():
                if not _vbias_box:
                    ones_row = const.tile([1, P], BF16, tag="ones",
                                          name="ones_row")
                    nc.vector.memset(ones_row[:], 1.0)
                    bv_row = const.tile([1, P], BF16, tag="bvr",
                                        name="bv_row")
                    nc.gpsimd.dma_start(
                        bv_row[:],
                        dr_b["v"].ap().rearrange("(o e) -> o e", o=1))
                    # bv tiled 4x so one rank-1 matmul biases a whole
                    # 4-strip bank group
                    bv4 = const.tile([1, 4 * P], BF16, tag="bv4",
                                     name="bv4")
                    for i in range(4):
                        nc.vector.tensor_copy(
                            bv4[:, i * P:(i + 1) * P], bv_row[:])
                    _vbias_box.append((ones_row, bv4))
                return _vbias_box[0]

            def load_chunk(t, b, cc, split=False):
                """One 1MB double D-chunk [128, 2, S] of input t.
                split=True issues the two slabs as separate DMAs (own
                semaphores) so the first projection matmul waits on
                512KB, not 1MB — used for the kernel's very first chunk
                where DMA latency is fully exposed."""
                x = streama.tile([P, 2, S], BF16, tag="stream", name="x")
                if split:
                    for two in range(2):
                        nc.gpsimd.dma_start(
                            x[:, two, :],
                            dr_in[t].ap()[b, (cc * 2 + two) * P:
                                          (cc * 2 + two + 1) * P, :])
                else:
                    nc.gpsimd.dma_start(
                        x[:],
                        dr_in[t].ap()[b, cc * 2 * P:(cc + 1) * 2 * P, :]
                        .rearrange("(two p) s -> p two s", two=2))
                ensure_w(t)
                return x

            def emit_qt_chunk(b, cc, halves, split=False):
                """One double-chunk of the Q projection."""
                x = load_chunk("q", b, cc, split=split)
                for two in range(2):
                    c = cc * 2 + two
                    for h in range(2):
                        for s2 in range(2):
                            nc.tensor.matmul(
                                halves[h][:, s2 * 512:(s2 + 1) * 512],
                                lhsT=w_sb["q"][:, c, :],
                                rhs=x[:, two, h * 1024 + s2 * 512:
                                      h * 1024 + (s2 + 1) * 512],
                                start=(c == 0), stop=(c == DC - 1))

            def emit_qt_finish(b, halves):
                out = proj.tile([P, S], BF16, tag="qT", name="qT")
                for h in range(2):
                    nc.vector.tensor_scalar_add(
                        out[:, h * 1024:(h + 1) * 1024],
                        halves[h][:], b_sb["q"][:])
                return out

            def emit_qt(b):
                """Q projection: 4 double-chunks -> [d, S] bf16."""
                halves = [ps_big.tile([P, 1024], F32, tag="big",
                                      name="q_ps") for _ in range(2)]
                for cc in range(DC // 2):
                    emit_qt_chunk(b, cc, halves,
                                  split=(b == 0 and cc == 0))
                return emit_qt_finish(b, halves)

            def emit_kslab(b, sl):
                """K super-chunk: one [D, 256] slab -> kct [d, 256] bf16
                (2 k-chunks worth of KT).  The accumulator rotates
                through a dedicated 2-bank PSUM pair so slab sl+1's
                projection never waits on slab sl's kct copy."""
                ensure_w("k")
                xk = streamk.tile([P, DC, 256], BF16, tag="streamk",
                                  name="xk")
                nc.gpsimd.dma_start(
                    xk[:],
                    dr_in["k"].ap()[b, :, sl * 256:(sl + 1) * 256]
                    .rearrange("(c p) s -> p c s", p=P))
                kpt = ps_kps.tile([P, 512], F32, tag="kps", name="kps")
                kps = kpt[:, 0:256]
                for c in range(DC):
                    nc.tensor.matmul(
                        kps, lhsT=w_sb["k"][:, c, :], rhs=xk[:, c, :],
                        start=(c == 0), stop=(c == DC - 1))
                kct = kctp.tile([P, 256], BF16, tag="kt", name="kct")
                nc.vector.tensor_scalar_add(kct[:], kps, b_sb["k"][:])
                return kct

            def emit_scores(qt, lhsT_ap, accum, after=(None, None)):
                """One k-chunk of scores^T + exp.  accum=True also
                row-sums via the ACT accumulator (used for the trailing
                chunks so AV isn't gated on late DVE reduces).
                after[h], if set, is emitted right after half h — fill
                work placed INSIDE the in-order PE stream where the
                chain stalls on the exp double-buffer."""
                pt = ptp.tile([P, S], BF16, tag="pt", name="pt")
                zz = zzp.tile([P, 2], F32, tag="z", name="zz") if accum \
                    else None
                for h in range(2):
                    sc = ps_big.tile([P, 1024], F32, tag="big",
                                     name="sc_ps")
                    for s2 in range(2):
                        nc.tensor.matmul(
                            sc[:, s2 * 512:(s2 + 1) * 512],
                            lhsT=lhsT_ap,
                            rhs=qt[:, h * 1024 + s2 * 512:
                                   h * 1024 + (s2 + 1) * 512],
                            start=True, stop=True)
                    if accum:
                        nc.scalar.activation(
                            pt[:, h * 1024:(h + 1) * 1024], sc[:],
                            func=mybir.ActivationFunctionType.Exp,
                            scale=SCALE, accum_out=zz[:, h:h + 1])
                    else:
                        nc.scalar.activation(
                            pt[:, h * 1024:(h + 1) * 1024], sc[:],
                            func=mybir.ActivationFunctionType.Exp,
                            scale=SCALE)
                    if after[h] is not None:
                        after[h]()
                return pt, zz

            def emit_rec_pt(pt):
                """1/Z from a DVE row-sum of the (bf16) exp tile —
                cheaper than ACT accum-readout, and off the ACT critical
                path.  Emitted at lag-4 behind its exp so the reduce
                never queues the DVE behind an in-flight EXP."""
                rec = recp.tile([P, 1], F32, tag="rec", name="rec")
                nc.vector.tensor_reduce(
                    rec[:], pt[:], axis=mybir.AxisListType.X,
                    op=mybir.AluOpType.add)
                nc.vector.reciprocal(rec[:], rec[:])
                return rec

            def emit_rec_zz(zz):
                rec = recp.tile([P, 1], F32, tag="rec", name="rec")
                nc.vector.tensor_reduce(
                    rec[:], zz[:], axis=mybir.AxisListType.X,
                    op=mybir.AluOpType.add)
                nc.vector.reciprocal(rec[:], rec[:])
                return rec

            def emit_vs(v_sb, kc, rec):
                vs = vsp.tile([P, P], BF16, tag="vs", name="vs")
                nc.vector.tensor_scalar_mul(
                    vs[:], v_sb[:, kc, :], rec[:])
                return vs

            def emit_vstrip_group(g, v_tiles, v_sb):
                """Four [128,128] V strips accumulated in ONE psum bank
                (natural [S, d] layout, stationary input slabs).  Only
                the bank's first write issues the clearing start; the
                other strips overwrite-on-first-write via the cleared
                has_written bits.  Needs all four V chunks resident —
                emitted inside the ACT-bound scores window."""
                ones_row, bv4 = ensure_vbias()
                ps = ps_out.tile([P, 4, P], F32, tag="out", name="vps")
                for s4 in range(4):
                    sc = g * 4 + s4
                    dst = ps[:, s4, :]
                    for cc in range(4):
                        for two in range(2):
                            c = cc * 2 + two
                            nc.tensor.matmul(
                                dst,
                                lhsT=v_tiles[cc][:, two,
                                                 sc * P:(sc + 1) * P],
                                rhs=w_sb["v"][:, c, :],
                                start=(s4 == 0 and c == 0), stop=False)
                # one bank-wide rank-1 bias matmul closes all four
                # strips' accumulation groups at once
                nc.tensor.matmul(
                    ps[:].rearrange("p a b -> p (a b)"),
                    lhsT=ones_row[:], rhs=bv4[:],
                    start=False, stop=True)
                nc.vector.tensor_copy(
                    v_sb[:, g * 4:(g + 1) * 4, :], ps[:])

            def emit_av_part(ps, st, vss, pts, kcs, first, last):
                """Part of one 512-col H^T strip accumulation.  Strips
                0/1 are split kc0-11 / kc12-15: the early parts' inputs
                are ready before the scores chain ends, so they fill the
                PE while the last exps drain on ACT."""
                sl = slice(st * 512, (st + 1) * 512)
                for i, kc in enumerate(kcs):
                    nc.tensor.matmul(
                        ps[:], lhsT=vss[kc][:], rhs=pts[kc][:, sl],
                        start=(first and i == 0),
                        stop=(last and i == len(kcs) - 1))

            def emit_av_flush(b, st, ps, out_sb):
                sl = slice(st * 512, (st + 1) * 512)
                nc.vector.tensor_copy(out_sb[:, sl], ps[:])
                nc.sync.dma_start(dr_out.ap()[b][:, sl], out_sb[:, sl])

            def emit_av_strip(b, st, vss, pts, out_sb):
                """One full H^T strip + flush."""
                ps = ps_out.tile([P, 512], F32, tag="out", name="avps")
                emit_av_part(ps, st, vss, pts, list(range(KC)),
                             True, True)
                emit_av_flush(b, st, ps, out_sb)

            def new_state(b):
                # One schedule for both batches: strip groups and AV
                # quanta sit late enough that their V data has provably
                # landed (earlier placements head-block the in-order PE
                # on in-flight DMAs and were measured bistable/slower).
                sched = {
                    "strips": {5: (0, 1, 2)}, "g3_top": 6,
                    "vss": {5: range(0, 4), 6: range(4, 8)},
                    "quanta": {12: ((0, 0, 4), (1, 0, 4)),
                               14: ((0, 4, 8), (1, 4, 8))},
                    "tailA": {0: (8, 12), 1: (8, 12),
                              2: (0, 12), 3: (0, 12)},
                }
                return {
                    "b": b, "sched": sched,
                    "v_sb": proj.tile([P, KC, P], BF16, tag="v",
                                      name="v"),
                    "v_tiles": [], "pts": [], "recs": {}, "vss": {},
                    "zz_h": {}, "kcts": [], "avps": [None] * 4,
                }

            def get_avps(s, st):
                """Lazily allocate strip st's AV psum bank: strips 0/1
                from ps_out (after the V strip groups), strips 2/3 from
                ps_kps (free once the last K slab is projected)."""
                if s["avps"][st] is None:
                    pool = ps_out if st < 2 else ps_kps
                    tag = "out" if st < 2 else "kps"
                    s["avps"][st] = pool.tile([P, 512], F32, tag=tag,
                                              name="avps")
                return s["avps"][st]

            def av_quantum(s, st, kcs):
                def fn():
                    emit_av_part(get_avps(s, st), st, s["vss"],
                                 s["pts"], kcs, kcs[0] == 0, False)
                return fn

            def emit_score_kc(s, kc, after=(None, None)):
                """One k-chunk of the scores/exp chain with lag-4 1/Z."""
                pt, zz = emit_scores(
                    s["qt"], s["kcts"][kc // 2][:, (kc % 2) * P:
                                                (kc % 2 + 1) * P],
                    accum=(kc >= KC - N_HYB), after=after)
                s["pts"].append(pt)
                if zz is not None:
                    s["zz_h"][kc] = zz
                if 4 <= kc and kc - 4 < KC - N_HYB:
                    s["recs"][kc - 4] = emit_rec_pt(s["pts"][kc - 4])

            def emit_slab_iter(s, sl):
                """Slab sl's two score chunks, pre-emitting slab sl+1's
                projection, the slotted V-chunk load, and the V strip
                groups once all of V is resident (spread sl5..7 so the
                PE consumes them inside the ACT-bound stretch)."""
                b = s["b"]
                if sl < 7 and len(s["kcts"]) == sl + 1:
                    s["kcts"].append(emit_kslab(b, sl + 1))
                v_slot = ({1: 0, 2: 1, 3: 2, 4: 3} if b == 0
                          else {0: 0, 1: 1, 2: 2, 3: 3})
                if sl in v_slot:
                    s["v_tiles"].append(load_chunk("v", b, v_slot[sl]))
                sched = s["sched"]
                if sl == sched["g3_top"]:
                    emit_vstrip_group(3, s["v_tiles"], s["v_sb"])
                for j in range(2):
                    kc = 2 * sl + j
                    after = (None, None)
                    # fill: AV quanta with provably-ready inputs dropped
                    # between the exp-gated score halves
                    if kc in sched["quanta"]:
                        qa, qb = sched["quanta"][kc]
                        after = (
                            av_quantum(s, qa[0],
                                       list(range(qa[1], qa[2]))),
                            av_quantum(s, qb[0],
                                       list(range(qb[1], qb[2]))))
                    emit_score_kc(s, kc, after)
                for g in sched["strips"].get(sl, ()):
                    emit_vstrip_group(g, s["v_tiles"], s["v_sb"])
                for kc in sched["vss"].get(sl, ()):
                    s["vss"][kc] = emit_vs(s["v_sb"], kc, s["recs"][kc])

            def emit_batch_tail(s):
                """Emitted right after slab 7: early AV parts (all four
                strips, kc0-11 — inputs all ready, strips 2/3 borrow the
                K-slab psum banks which are free once slab 7 is done)
                fill the PE while the last exps drain, then tail 1/Z +
                vs."""
                for kc in range(8, 12):
                    s["vss"][kc] = emit_vs(s["v_sb"], kc, s["recs"][kc])
                # finish each strip's partA: whatever the in-window
                # quanta didn't already accumulate
                for st in range(4):
                    lo, hi = s["sched"]["tailA"][st]
                    emit_av_part(get_avps(s, st), st, s["vss"],
                                 s["pts"], list(range(lo, hi)),
                                 lo == 0, False)
                for kc in range(KC - N_HYB, KC):
                    s["recs"][kc] = emit_rec_zz(s["zz_h"][kc])
                for kc in range(12, KC):
                    s["vss"][kc] = emit_vs(s["v_sb"], kc, s["recs"][kc])
                s["out_sb"] = osb.tile([P, S], BF16, tag="osb",
                                       name="out_sb")

            def emit_av_close(s, st):
                """Close strip st: the kc12-15 remainder + flush."""
                emit_av_part(s["avps"][st], st, s["vss"], s["pts"],
                             list(range(12, KC)), False, True)
                emit_av_flush(s["b"], st, s["avps"][st], s["out_sb"])

            # ---- two-batch software-pipelined emission.  The PE runs
            # instructions strictly in emission order, so batch 1's
            # DMA-gated prologue (q projection chunks, first K slabs +
            # score chunks) is staggered BETWEEN batch 0's AV strips:
            # each hoisted instruction is placed where its input data
            # has already landed, and batch 1's early exps keep the ACT
            # engine fed while the PE chews batch 0's AV matmuls. ----
            s0 = new_state(0)
            s0["qt"] = emit_qt(0)
            s0["kcts"].append(emit_kslab(0, 0))
            for sl in range(8):
                emit_slab_iter(s0, sl)
            emit_batch_tail(s0)

            s1 = new_state(1)
            halves1 = [ps_big.tile([P, 1024], F32, tag="big",
                                   name="q_ps") for _ in range(2)]
            # b1 q chunks 0-1 land right behind b0's input stream
            emit_qt_chunk(1, 0, halves1)
            emit_qt_chunk(1, 1, halves1)
            emit_av_close(s0, 0)
            emit_qt_chunk(1, 2, halves1)
            emit_av_close(s0, 1)
            emit_qt_chunk(1, 3, halves1)
            s1["qt"] = emit_qt_finish(1, halves1)
            # close s0's strips 2/3 BEFORE the b1 kslab that reuses the
            # same ps_kps bank: the slab's clearing start must not land
            # between a strip's partA and partB accumulations
            emit_av_close(s0, 2)
            s1["kcts"].append(emit_kslab(1, 0))
            emit_av_close(s0, 3)
            s1["kcts"].append(emit_kslab(1, 1))
            for sl in range(8):
                emit_slab_iter(s1, sl)
            emit_batch_tail(s1)
            for st in range(4):
                emit_av_close(s1, st)

    nc.compile()
    return nc


def _get_nc():
    global _BUILT
    if _BUILT is None:
        _BUILT = build()
    return _BUILT


def kernel(inp_q, inp_k, inp_v, Wq_kernel, Wq_bias, Wk_kernel, Wk_bias,
           Wv_kernel, Wv_bias):
    from concourse.bass_utils import run_bass_kernel_spmd

    nc = _get_nc()

    inp = {"q": np.asarray(inp_q, dtype=np.float32).astype(ml_dtypes.bfloat16),
           "k": np.asarray(inp_k, dtype=np.float32).astype(ml_dtypes.bfloat16),
           "v": np.asarray(inp_v, dtype=np.float32).astype(ml_dtypes.bfloat16)}
    w = {"q": np.ascontiguousarray(
             np.asarray(Wq_kernel, dtype=np.float32)
             .astype(ml_dtypes.bfloat16)),
         "k": np.ascontiguousarray(
             np.asarray(Wk_kernel, dtype=np.float32)
             .astype(ml_dtypes.bfloat16)),
         "v": np.ascontiguousarray(
             np.asarray(Wv_kernel, dtype=np.float32)
             .astype(ml_dtypes.bfloat16))}
    bias = {"q": np.ascontiguousarray(np.asarray(Wq_bias, dtype=np.float32)),
            "k": np.ascontiguousarray(np.asarray(Wk_bias, dtype=np.float32)),
            "v": np.ascontiguousarray(np.asarray(Wv_bias, dtype=np.float32))}

    in_maps = []
    for c in range(N_CORES):
        m = {}
        for t in ("q", "k", "v"):
            # [2, S, D] -> [2, D, S] contiguous (pure layout marshalling)
            m[f"{t}T"] = np.ascontiguousarray(
                inp[t][c * B_LOC:(c + 1) * B_LOC].transpose(0, 2, 1))
            m[f"w{t}"] = w[t]
            m[f"b{t}"] = bias[t]
        in_maps.append(m)

    res = run_bass_kernel_spmd(nc, in_maps, list(range(N_CORES)))

    out = np.empty((N_CORES * B_LOC, S, P), dtype=np.float32)
    for c in range(N_CORES):
        # [2, P, S] bf16 -> [2, S, P] f32
        out[c * B_LOC:(c + 1) * B_LOC] = (
            res.results[c]["out"].astype(np.float32).transpose(0, 2, 1))
    return out


# revision 54
# speedup vs baseline: 1.0247x; 1.0247x over previous
"""Trainium2 Bass kernel for the AttentionLayer problem.

Math (per batch):
    Q = inp_q @ Wq + bq            [S, d]
    K = inp_k @ Wk + bk            [S, d]
    V = inp_v @ Wv + bv            [S, d]
    sc = Q @ K^T / sqrt(d)         [Sq, Sk]
    S_ = softmax(sc, axis=0)       (over the QUERY axis)
    H = S_ @ V                     [Sq, d]

Device-side layout strategy (per core, 2 batches):
  * Host feeds transposed activations xT = x^T [D, S] in bf16 so every
    matmul contracts over the SBUF partition dim with zero on-chip
    transposes and minimal HBM traffic (compute is bf16 anyway).
  * Projections produce QT/KT in [d, S] layout (d = 128 partitions).
  * scores^T [k, q] = (KT-slice)^T @ QT, so softmax-over-q is a
    free-axis row reduction.  No max-subtraction is needed:
    |sc/sqrt(d)| <~ 6 for randn inputs, exp() is exact in f32 there.
  * The scores chain is ACT-bound (exp of S^2 elements), so all other
    PE work is interleaved INTO it: K-slab projections (double-buffered
    PSUM bank pair) and the V projection, computed in natural [S, d]
    layout as 4-strip bank groups once all four V chunks have landed.
  * Z[k] = sum_q exp is a DVE reduce over the bf16 pt tile for most
    chunks (cheaper than ACT's accum-readout); the last 4 chunks keep
    the ACT accumulator so the AV phase isn't gated on tail reduces.
  * Normalization is folded into V: vs[k, :] = V[k, :] / Z[k], then
    H^T [d, q] += vs-slice^T @ P^T accumulates per 512-col strip in a
    single PSUM bank; each strip converts to bf16 and DMAs out while
    the next strip's matmuls run.
  * PSUM budget: scores double-buffer 2x[128,1024] (4 banks) + K-slab
    pair (2 banks) + V/AV strip pair (2 banks) = 8 banks.
  * DMA triggers: all input loads on gpsimd in program order (the DMA
    bus is FIFO by trigger time; one queue keeps big transfers from
    cutting ahead of later-needed slabs), weights on scalar, biases +
    output on sync.  Descriptor generation costs ~1us fixed per
    dma_start, so parallel queues matter at startup.
  * Output H^T stored bf16; host upcasts + un-transposes.
Compute dtype bf16 (f32 PSUM accumulate), stats in f32.
"""

import math
import sys

sys.path.insert(0, "/opt/trn_rl_repo")

import ml_dtypes
import numpy as np

import concourse.bass as bass  # noqa: E402
import concourse.tile as tile  # noqa: E402
from concourse import bacc, mybir  # noqa: E402

P = 128          # partitions / head dim d
S = 2048         # sequence length
D = 1024         # model dim
DC = D // P      # D chunks (8)
KC = S // P      # key chunks (16)
B_LOC = 2        # batches per core
N_CORES = 8
SCALE = 1.0 / math.sqrt(P)
N_HYB = 4        # trailing k-chunks whose Z uses the ACT accumulator

F32 = mybir.dt.float32
BF16 = mybir.dt.bfloat16

_BUILT = None  # cached (nc,) so repeated kernel() calls reuse the NEFF


def build():
    nc = bacc.Bacc("TRN2", target_bir_lowering=False, debug=False,
                   num_devices=N_CORES)

    dr_in = {}
    for t in ("q", "k", "v"):
        dr_in[t] = nc.dram_tensor(f"{t}T", [B_LOC, D, S], BF16,
                                  kind="ExternalInput")
    dr_w = {t: nc.dram_tensor(f"w{t}", [D, P], BF16, kind="ExternalInput")
            for t in ("q", "k", "v")}
    dr_b = {t: nc.dram_tensor(f"b{t}", [P], F32, kind="ExternalInput")
            for t in ("q", "k", "v")}
    dr_out = nc.dram_tensor("out", [B_LOC, P, S], BF16,
                            kind="ExternalOutput")

    with tile.TileContext(nc) as tc:
        with (
            tc.tile_pool(name="const", bufs=1) as const,
            tc.tile_pool(name="streama", bufs=8) as streama,
            tc.tile_pool(name="streamk", bufs=8) as streamk,
            tc.tile_pool(name="proj", bufs=2) as proj,
            tc.tile_pool(name="kctp", bufs=10) as kctp,
            tc.tile_pool(name="ptp", bufs=16) as ptp,
            tc.tile_pool(name="vsp", bufs=18) as vsp,
            tc.tile_pool(name="recp", bufs=18) as recp,
            tc.tile_pool(name="zzp", bufs=6) as zzp,
            tc.tile_pool(name="osb", bufs=1) as osb,
            tc.tile_pool(name="ps_big", bufs=2, space="PSUM") as ps_big,
            tc.tile_pool(name="ps_kps", bufs=2, space="PSUM") as ps_kps,
            tc.tile_pool(name="ps_out", bufs=2, space="PSUM") as ps_out,
        ):
            w_sb = {}
            b_sb = {}
            _w_loaded = set()

            def ensure_w(t):
                if t in _w_loaded:
                    return
                _w_loaded.add(t)
                nc.scalar.dma_start(
                    w_sb[t][:],
                    dr_w[t].ap().rearrange("(c p) e -> p c e", p=P))

            for t in ("q", "k", "v"):
                w_sb[t] = const.tile([P, DC, P], BF16, tag=f"w{t}",
                                     name=f"w{t}")
                b_sb[t] = const.tile([P, 1], F32, tag=f"b{t}", name=f"b{t}")
                nc.sync.dma_start(
                    b_sb[t][:],
                    dr_b[t].ap().rearrange("(p o) -> p o", o=1))
            # V bias as a rank-1 matmul (ones[1,128].T @ bias_row[1,128])
            # closing each V strip's accumulation group; created lazily
            _vbias_box = []

            def ensure_vbias():
                if not _vbias_box:
                    ones_row = const.tile([1, P], BF16, tag="ones",
                                          name="ones_row")
                    nc.vector.memset(ones_row[:], 1.0)
                    bv_row = const.tile([1, P], BF16, tag="bvr",
                                        name="bv_row")
                    nc.gpsimd.dma_start(
                        bv_row[:],
                        dr_b["v"].ap().rearrange("(o e) -> o e", o=1))
                    # bv tiled 4x so one rank-1 matmul biases a whole
                    # 4-strip bank group
                    bv4 = const.tile([1, 4 * P], BF16, tag="bv4",
                                     name="bv4")
                    for i in range(4):
                        nc.vector.tensor_copy(
                            bv4[:, i * P:(i + 1) * P], bv_row[:])
                    _vbias_box.append((ones_row, bv4))
                return _vbias_box[0]

            def load_chunk(t, b, cc, split=False):
                """One 1MB double D-chunk [128, 2, S] of input t.
                split=True issues the two slabs as separate DMAs (own
                semaphores) so the first projection matmul waits on
                512KB, not 1MB — used for the kernel's very first chunk
                where DMA latency is fully exposed."""
                x = streama.tile([P, 2, S], BF16, tag="stream", name="x")
                if split:
                    for two in range(2):
                        nc.gpsimd.dma_start(
                            x[:, two, :],
                            dr_in[t].ap()[b, (cc * 2 + two) * P:
                                          (cc * 2 + two + 1) * P, :])
                else:
                    nc.gpsimd.dma_start(
                        x[:],
                        dr_in[t].ap()[b, cc * 2 * P:(cc + 1) * 2 * P, :]
                        .rearrange("(two p) s -> p two s", two=2))
                ensure_w(t)
                return x

            def emit_qt_chunk(b, cc, halves, split=False):
                """One double-chunk of the Q projection."""
                x = load_chunk("q", b, cc, split=split)
                for two in range(2):
                    c = cc * 2 + two
                    for h in range(2):
                        for s2 in range(2):
                            nc.tensor.matmul(
                                halves[h][:, s2 * 512:(s2 + 1) * 512],
                                lhsT=w_sb["q"][:, c, :],
                                rhs=x[:, two, h * 1024 + s2 * 512:
                                      h * 1024 + (s2 + 1) * 512],
                                start=(c == 0), stop=(c == DC - 1))

            def emit_qt_finish(b, halves):
                out = proj.tile([P, S], BF16, tag="qT", name="qT")
                for h in range(2):
                    nc.vector.tensor_scalar_add(
                        out[:, h * 1024:(h + 1) * 1024],
                        halves[h][:], b_sb["q"][:])
                return out

            def emit_qt(b):
                """Q projection: 4 double-chunks -> [d, S] bf16."""
                halves = [ps_big.tile([P, 1024], F32, tag="big",
                                      name="q_ps") for _ in range(2)]
                for cc in range(DC // 2):
                    emit_qt_chunk(b, cc, halves,
                                  split=(b == 0 and cc == 0))
                return emit_qt_finish(b, halves)

            def emit_kslab(b, sl):
                """K super-chunk: one [D, 256] slab -> kct [d, 256] bf16
                (2 k-chunks worth of KT).  The accumulator rotates
                through a dedicated 2-bank PSUM pair so slab sl+1's
                projection never waits on slab sl's kct copy."""
                ensure_w("k")
                xk = streamk.tile([P, DC, 256], BF16, tag="streamk",
                                  name="xk")
                nc.gpsimd.dma_start(
                    xk[:],
                    dr_in["k"].ap()[b, :, sl * 256:(sl + 1) * 256]
                    .rearrange("(c p) s -> p c s", p=P))
                kpt = ps_kps.tile([P, 512], F32, tag="kps", name="kps")
                kps = kpt[:, 0:256]
                for c in range(DC):
                    nc.tensor.matmul(
                        kps, lhsT=w_sb["k"][:, c, :], rhs=xk[:, c, :],
                        start=(c == 0), stop=(c == DC - 1))
                kct = kctp.tile([P, 256], BF16, tag="kt", name="kct")
                nc.vector.tensor_scalar_add(kct[:], kps, b_sb["k"][:])
                return kct

            def emit_scores(qt, lhsT_ap, accum, after=(None, None)):
                """One k-chunk of scores^T + exp.  accum=True also
                row-sums via the ACT accumulator (used for the trailing
                chunks so AV isn't gated on late DVE reduces).
                after[h], if set, is emitted right after half h — fill
                work placed INSIDE the in-order PE stream where the
                chain stalls on the exp double-buffer."""
                pt = ptp.tile([P, S], BF16, tag="pt", name="pt")
                zz = zzp.tile([P, 2], F32, tag="z", name="zz") if accum \
                    else None
                for h in range(2):
                    sc = ps_big.tile([P, 1024], F32, tag="big",
                                     name="sc_ps")
                    for s2 in range(2):
                        nc.tensor.matmul(
                            sc[:, s2 * 512:(s2 + 1) * 512],
                            lhsT=lhsT_ap,
                            rhs=qt[:, h * 1024 + s2 * 512:
                                   h * 1024 + (s2 + 1) * 512],
                            start=True, stop=True)
                    if accum:
                        nc.scalar.activation(
                            pt[:, h * 1024:(h + 1) * 1024], sc[:],
                            func=mybir.ActivationFunctionType.Exp,
                            scale=SCALE, accum_out=zz[:, h:h + 1])
                    else:
                        nc.scalar.activation(
                            pt[:, h * 1024:(h + 1) * 1024], sc[:],
                            func=mybir.ActivationFunctionType.Exp,
                            scale=SCALE)
                    if after[h] is not None:
                        after[h]()
                return pt, zz

            def emit_rec_pt(pt):
                """1/Z from a DVE row-sum of the (bf16) exp tile —
                cheaper than ACT accum-readout, and off the ACT critical
                path.  Emitted at lag-4 behind its exp so the reduce
                never queues the DVE behind an in-flight EXP."""
                rec = recp.tile([P, 1], F32, tag="rec", name="rec")
                nc.vector.tensor_reduce(
                    rec[:], pt[:], axis=mybir.AxisListType.X,
                    op=mybir.AluOpType.add)
                nc.vector.reciprocal(rec[:], rec[:])
                return rec

            def emit_rec_zz(zz):
                rec = recp.tile([P, 1], F32, tag="rec", name="rec")
                nc.vector.tensor_reduce(
                    rec[:], zz[:], axis=mybir.AxisListType.X,
                    op=mybir.AluOpType.add)
                nc.vector.reciprocal(rec[:], rec[:])
                return rec

            def emit_vs(v_sb, kc, rec):
                vs = vsp.tile([P, P], BF16, tag="vs", name="vs")
                nc.vector.tensor_scalar_mul(
                    vs[:], v_sb[:, kc, :], rec[:])
                return vs

            def emit_vstrip_group(g, v_tiles, v_sb):
                """Four [128,128] V strips accumulated in ONE psum bank
                (natural [S, d] layout, stationary input slabs).  Only
                the bank's first write issues the clearing start; the
                other strips overwrite-on-first-write via the cleared
                has_written bits.  Needs all four V chunks resident —
                emitted inside the ACT-bound scores window."""
                ones_row, bv4 = ensure_vbias()
                ps = ps_out.tile([P, 4, P], F32, tag="out", name="vps")
                for s4 in range(4):
                    sc = g * 4 + s4
                    dst = ps[:, s4, :]
                    for cc in range(4):
                        for two in range(2):
                            c = cc * 2 + two
                            nc.tensor.matmul(
                                dst,
                                lhsT=v_tiles[cc][:, two,
                                                 sc * P:(sc + 1) * P],
                                rhs=w_sb["v"][:, c, :],
                                start=(s4 == 0 and c == 0), stop=False)
                # one bank-wide rank-1 bias matmul closes all four
                # strips' accumulation groups at once
                nc.tensor.matmul(
                    ps[:].rearrange("p a b -> p (a b)"),
                    lhsT=ones_row[:], rhs=bv4[:],
                    start=False, stop=True)
                nc.vector.tensor_copy(
                    v_sb[:, g * 4:(g + 1) * 4, :], ps[:])

            def emit_av_part(ps, st, vss, pts, kcs, first, last):
                """Part of one 512-col H^T strip accumulation.  Strips
                0/1 are split kc0-11 / kc12-15: the early parts' inputs
                are ready before the scores chain ends, so they fill the
                PE while the last exps drain on ACT."""
                sl = slice(st * 512, (st + 1) * 512)
                for i, kc in enumerate(kcs):
                    nc.tensor.matmul(
                        ps[:], lhsT=vss[kc][:], rhs=pts[kc][:, sl],
                        start=(first and i == 0),
                        stop=(last and i == len(kcs) - 1))

            def emit_av_flush(b, st, ps, out_sb):
                sl = slice(st * 512, (st + 1) * 512)
                nc.vector.tensor_copy(out_sb[:, sl], ps[:])
                nc.sync.dma_start(dr_out.ap()[b][:, sl], out_sb[:, sl])

            def emit_av_strip(b, st, vss, pts, out_sb):
                """One full H^T strip + flush."""
                ps = ps_out.tile([P, 512], F32, tag="out", name="avps")
                emit_av_part(ps, st, vss, pts, list(range(KC)),
                             True, True)
                emit_av_flush(b, st, ps, out_sb)

            def new_state(b):
                # Strip groups and AV quanta sit late enough that their
                # V data has provably landed (earlier placements
                # head-block the in-order PE on in-flight DMAs and were
                # measured bistable/slower).  Batch 0's strip-2/3 partA
                # moves into the transition (interleaved with batch 1's
                # first score chunks so ACT restarts early); batch 1's
                # early V loads are emitted in the transition too.
                sched = {
                    "strips": {5: (0, 1, 2)}, "g3_top": 6,
                    "vss": {5: range(0, 4), 6: range(4, 8)},
                    "quanta": {12: ((0, 0, 4), (1, 0, 4)),
                               14: ((0, 4, 8), (1, 4, 8))},
                    "tailA": ({0: (8, 12), 1: (8, 12)} if b == 0 else
                              {0: (8, 12), 1: (8, 12),
                               2: (0, 12), 3: (0, 12)}),
                    "v_slot": ({1: 0, 2: 1, 3: 2, 4: 3} if b == 0
                               else {3: 3}),
                }
                return {
                    "b": b, "sched": sched,
                    "v_sb": proj.tile([P, KC, P], BF16, tag="v",
                                      name="v"),
                    "v_tiles": [], "pts": [], "recs": {}, "vss": {},
                    "zz_h": {}, "kcts": [], "avps": [None] * 4,
                }

            def get_avps(s, st):
                """Lazily allocate strip st's AV psum bank: strips 0/1
                from ps_out (after the V strip groups), strips 2/3 from
                ps_kps (free once the last K slab is projected)."""
                if s["avps"][st] is None:
                    pool = ps_out if st < 2 else ps_kps
                    tag = "out" if st < 2 else "kps"
                    s["avps"][st] = pool.tile([P, 512], F32, tag=tag,
                                              name="avps")
                return s["avps"][st]

            def av_quantum(s, st, kcs):
                def fn():
                    emit_av_part(get_avps(s, st), st, s["vss"],
                                 s["pts"], kcs, kcs[0] == 0, False)
                return fn

            def emit_score_kc(s, kc, after=(None, None)):
                """One k-chunk of the scores/exp chain with lag-4 1/Z."""
                pt, zz = emit_scores(
                    s["qt"], s["kcts"][kc // 2][:, (kc % 2) * P:
                                                (kc % 2 + 1) * P],
                    accum=(kc >= KC - N_HYB), after=after)
                s["pts"].append(pt)
                if zz is not None:
                    s["zz_h"][kc] = zz
                if 4 <= kc and kc - 4 < KC - N_HYB:
                    s["recs"][kc - 4] = emit_rec_pt(s["pts"][kc - 4])

            def emit_slab_iter(s, sl):
                """Slab sl's two score chunks, pre-emitting slab sl+1's
                projection, the slotted V-chunk load, and the V strip
                groups once all of V is resident (spread sl5..7 so the
                PE consumes them inside the ACT-bound stretch)."""
                b = s["b"]
                if sl < 7 and len(s["kcts"]) == sl + 1:
                    s["kcts"].append(emit_kslab(b, sl + 1))
                sched = s["sched"]
                if sl in sched["v_slot"]:
                    s["v_tiles"].append(
                        load_chunk("v", b, sched["v_slot"][sl]))
                if sl == sched["g3_top"]:
                    emit_vstrip_group(3, s["v_tiles"], s["v_sb"])
                for j in range(2):
                    kc = 2 * sl + j
                    after = (None, None)
                    # fill: AV quanta with provably-ready inputs dropped
                    # between the exp-gated score halves
                    if kc in sched["quanta"]:
                        qa, qb = sched["quanta"][kc]
                        after = (
                            av_quantum(s, qa[0],
                                       list(range(qa[1], qa[2]))),
                            av_quantum(s, qb[0],
                                       list(range(qb[1], qb[2]))))
                    emit_score_kc(s, kc, after)
                for g in sched["strips"].get(sl, ()):
                    emit_vstrip_group(g, s["v_tiles"], s["v_sb"])
                for kc in sched["vss"].get(sl, ()):
                    s["vss"][kc] = emit_vs(s["v_sb"], kc, s["recs"][kc])

            def emit_batch_tail(s):
                """Emitted right after slab 7: early AV parts (all four
                strips, kc0-11 — inputs all ready, strips 2/3 borrow the
                K-slab psum banks which are free once slab 7 is done)
                fill the PE while the last exps drain, then tail 1/Z +
                vs."""
                for kc in range(8, 12):
                    s["vss"][kc] = emit_vs(s["v_sb"], kc, s["recs"][kc])
                # finish each strip's partA: whatever the in-window
                # quanta didn't already accumulate
                for st, (lo, hi) in sorted(s["sched"]["tailA"].items()):
                    emit_av_part(get_avps(s, st), st, s["vss"],
                                 s["pts"], list(range(lo, hi)),
                                 lo == 0, False)
                for kc in range(KC - N_HYB, KC):
                    s["recs"][kc] = emit_rec_zz(s["zz_h"][kc])
                for kc in range(12, KC):
                    s["vss"][kc] = emit_vs(s["v_sb"], kc, s["recs"][kc])
                s["out_sb"] = osb.tile([P, S], BF16, tag="osb",
                                       name="out_sb")

            def emit_av_close(s, st):
                """Close strip st: the kc12-15 remainder + flush."""
                emit_av_part(s["avps"][st], st, s["vss"], s["pts"],
                             list(range(12, KC)), False, True)
                emit_av_flush(s["b"], st, s["avps"][st], s["out_sb"])

            # ---- two-batch software-pipelined emission.  The PE runs
            # instructions strictly in emission order, so batch 1's
            # DMA-gated prologue (q projection chunks, first K slabs +
            # score chunks) is staggered BETWEEN batch 0's AV strips:
            # each hoisted instruction is placed where its input data
            # has already landed, and batch 1's early exps keep the ACT
            # engine fed while the PE chews batch 0's AV matmuls. ----
            s0 = new_state(0)
            s0["qt"] = emit_qt(0)
            s0["kcts"].append(emit_kslab(0, 0))
            for sl in range(8):
                emit_slab_iter(s0, sl)
            emit_batch_tail(s0)

            s1 = new_state(1)
            halves1 = [ps_big.tile([P, 1024], F32, tag="big",
                                   name="q_ps") for _ in range(2)]
            # b1 q chunks land right behind b0's input stream; b1's
            # first six score chunks are hoisted between b0's remaining
            # AV work so the ACT engine restarts on b1's exps ~25us
            # earlier instead of starving behind a bunched PE tail
            emit_qt_chunk(1, 0, halves1)
            emit_qt_chunk(1, 1, halves1)
            emit_av_close(s0, 0)
            emit_qt_chunk(1, 2, halves1)
            emit_av_close(s0, 1)
            emit_qt_chunk(1, 3, halves1)
            s1["qt"] = emit_qt_finish(1, halves1)
            s1["kcts"].append(emit_kslab(1, 0))
            s1["kcts"].append(emit_kslab(1, 1))
            emit_score_kc(s1, 0)
            # b0 strip 2/3 partA borrows the ps_kps rotation — emitted
            # between b1's score chunks; each strip closes before the
            # next b1 kslab that would recycle its bank
            emit_av_part(get_avps(s0, 2), 2, s0["vss"], s0["pts"],
                         list(range(12)), True, False)
            emit_score_kc(s1, 1)
            emit_av_close(s0, 2)
            s1["kcts"].append(emit_kslab(1, 2))
            s1["v_tiles"].append(load_chunk("v", 1, 0))
            emit_score_kc(s1, 2)
            emit_score_kc(s1, 3)
            emit_av_part(get_avps(s0, 3), 3, s0["vss"], s0["pts"],
                         list(range(12)), True, False)
            s1["kcts"].append(emit_kslab(1, 3))
            s1["v_tiles"].append(load_chunk("v", 1, 1))
            emit_score_kc(s1, 4)
            emit_score_kc(s1, 5)
            emit_av_close(s0, 3)
            s1["v_tiles"].append(load_chunk("v", 1, 2))
            for sl in range(3, 8):
                emit_slab_iter(s1, sl)
            emit_batch_tail(s1)
            for st in range(4):
                emit_av_close(s1, st)

    nc.compile()
    return nc


def _get_nc():
    global _BUILT
    if _BUILT is None:
        _BUILT = build()
    return _BUILT


def kernel(inp_q, inp_k, inp_v, Wq_kernel, Wq_bias, Wk_kernel, Wk_bias,
           Wv_kernel, Wv_bias):
    from concourse.bass_utils import run_bass_kernel_spmd

    nc = _get_nc()

    inp = {"q": np.asarray(inp_q, dtype=np.float32).astype(ml_dtypes.bfloat16),
           "k": np.asarray(inp_k, dtype=np.float32).astype(ml_dtypes.bfloat16),
           "v": np.asarray(inp_v, dtype=np.float32).astype(ml_dtypes.bfloat16)}
    w = {"q": np.ascontiguousarray(
             np.asarray(Wq_kernel, dtype=np.float32)
             .astype(ml_dtypes.bfloat16)),
         "k": np.ascontiguousarray(
             np.asarray(Wk_kernel, dtype=np.float32)
             .astype(ml_dtypes.bfloat16)),
         "v": np.ascontiguousarray(
             np.asarray(Wv_kernel, dtype=np.float32)
             .astype(ml_dtypes.bfloat16))}
    bias = {"q": np.ascontiguousarray(np.asarray(Wq_bias, dtype=np.float32)),
            "k": np.ascontiguousarray(np.asarray(Wk_bias, dtype=np.float32)),
            "v": np.ascontiguousarray(np.asarray(Wv_bias, dtype=np.float32))}

    in_maps = []
    for c in range(N_CORES):
        m = {}
        for t in ("q", "k", "v"):
            # [2, S, D] -> [2, D, S] contiguous (pure layout marshalling)
            m[f"{t}T"] = np.ascontiguousarray(
                inp[t][c * B_LOC:(c + 1) * B_LOC].transpose(0, 2, 1))
            m[f"w{t}"] = w[t]
            m[f"b{t}"] = bias[t]
        in_maps.append(m)

    res = run_bass_kernel_spmd(nc, in_maps, list(range(N_CORES)))

    out = np.empty((N_CORES * B_LOC, S, P), dtype=np.float32)
    for c in range(N_CORES):
        # [2, P, S] bf16 -> [2, S, P] f32
        out[c * B_LOC:(c + 1) * B_LOC] = (
            res.results[c]["out"].astype(np.float32).transpose(0, 2, 1))
    return out
